# revision 16
# baseline (speedup 1.0000x reference)
"""MoCo loss (InfoNCE over a 65536-entry queue + proto-NCE over 50000
k-means centroids) on 8 Trainium2 NeuronCores.

Strategy: the heavy work is two matmuls, Z_q @ queue.T (256x512x65536)
and Z_q @ centroids.T (256x512x50000).  We shard the tables by row
across the 8 cores and replicate Z_q.  The host pre-transposes the
tables to [C, rows] (fp16) so both matmul operands have the contraction
dim C on partitions.  Per core the device computes:

  part 1 (queue shard, layout [queue rows -> partitions, batch -> free]):
    - s1 = q_shard @ Z_q.T                      (PE, fp16 in / fp32 acc)
    - row-max over batch + (s1[:,0] >= rowmax)  -> accuracy counts (DVE)
    - exp(s1/T - SHIFT)                         (ACT)
    - ones-matmul partition sum                 -> per-batch partial
                                                   sum-of-exp (PE)
  part 2 (centroid shard, layout [batch -> partitions, centroids -> free]):
    - s2 = Z_q @ c_shard.T                      (PE)
    - top-1 value+index per batch row           (DVE max/max_index)
      (centroids are L2-normalized so argmin ||c||^2 - 2 s == argmax s)
    - s2 shard exported to DRAM

The host combines the tiny per-core partials (logsumexp merge, accuracy
count, global argmax) and does the 256x512 exclusion gather + 513-wide
softmax for the proto term (0.003% of total FLOPs).
"""

import os
import numpy as np

B, C = 256, 512
QUEUE, NCL, NNEG = 65536, 50000, 512
INFO_TEMP = 0.07
PROTO_FACTOR = 0.5
NCORES = 8
QSH = QUEUE // NCORES          # 8192 queue rows per core
CSH = NCL // NCORES            # 6250 centroid rows per core
CSH_PAD = 6272                 # 49 * 128
KSUB = C // 128                # 4 contraction subtiles
SHIFT = 14.0                   # logits <= 1/0.07 = 14.2857
NJT = QSH // 128               # 64 part-1 tiles
QCHUNK = 8                     # part-1 DMA chunks (1 MiB each)

_CACHE = {}

# exec time of the last device run (ns), populated when tracing is on
last_exec_time_ns = None


def _build(p1=True, p2=True, ones=True, rearrange=True, lpos=True):
    import concourse.bass as bass
    import concourse.tile as tile
    from concourse import bacc, mybir

    dt = mybir.dt
    nc = bacc.Bacc(
        "TRN2", target_bir_lowering=False, debug=False, num_devices=NCORES
    )

    # ---- DRAM I/O ----
    zqT_d = nc.dram_tensor("zqT", [KSUB, 128, B], dt.float16, kind="ExternalInput").ap()
    qT_d = nc.dram_tensor(
        "qT", [QCHUNK, KSUB, 128, QSH // QCHUNK], dt.float16, kind="ExternalInput"
    ).ap()
    cT_d = nc.dram_tensor(
        "cT", [KSUB, 128, CSH_PAD], dt.float16, kind="ExternalInput"
    ).ap()
    zq_d = nc.dram_tensor("zq", [2, 128, C], dt.float32, kind="ExternalInput").ap()
    zk_d = nc.dram_tensor("zk", [2, 128, C], dt.float32, kind="ExternalInput").ap()

    # [128, B]: all rows identical; a single-partition-row DMA fails NEFF load
    # on this runtime, so ship the whole tile and read row 0 on the host
    p1sum_d = nc.dram_tensor("p1sum", [128, B], dt.float32, kind="ExternalOutput").ap()
    p1acc_d = nc.dram_tensor("p1acc", [128], dt.float32, kind="ExternalOutput").ap()
    s2_d = nc.dram_tensor(
        "s2", [2, 128, CSH_PAD], dt.float32, kind="ExternalOutput"
    ).ap()
    p2max_d = nc.dram_tensor("p2max", [2, 128], dt.float32, kind="ExternalOutput").ap()
    p2idx_d = nc.dram_tensor("p2idx", [2, 128], dt.uint32, kind="ExternalOutput").ap()
    lpos_d = nc.dram_tensor("lpos", [2, 128], dt.float32, kind="ExternalOutput").ap()

    JW = QSH // QCHUNK  # 1024 columns per qT chunk
    with tile.TileContext(nc) as tc:
        with (
            tc.tile_pool(name="const", bufs=1) as cpool,
            tc.tile_pool(name="work", bufs=4) as wpool,
            tc.tile_pool(name="ps1", bufs=4, space="PSUM") as ps1,
            tc.tile_pool(name="psum1s", bufs=1, space="PSUM") as ps1s,
            tc.tile_pool(name="ps2", bufs=3, space="PSUM") as ps2,
        ):
            # ---- resident SBUF tensors ----
            zqT_sb = cpool.tile([128, KSUB, B], dt.float16)
            for s in range(KSUB):
                nc.sync.dma_start(zqT_sb[:, s, :], zqT_d[s])

            ones_sb = cpool.tile([128, 128], dt.bfloat16)
            nc.vector.memset(ones_sb[:], 1.0)
            nshift_sb = cpool.tile([128, 1], dt.float32)
            nc.vector.memset(nshift_sb[:], -SHIFT)

            qt_sb = []
            for h in range(QCHUNK):
                t = cpool.tile([128, KSUB, JW], dt.float16, tag=f"qt{h}")
                if rearrange:
                    nc.sync.dma_start(t[:], qT_d[h].rearrange("s c j -> c s j"))
                else:
                    for s in range(KSUB):
                        nc.sync.dma_start(t[:, s, :], qT_d[h, s])
                qt_sb.append(t)

            cT_sb = cpool.tile([128, KSUB, CSH_PAD], dt.float16)
            for s in range(KSUB):
                nc.sync.dma_start(cT_sb[:, s, :], cT_d[s])

            zq_sb = cpool.tile([128, 2, C], dt.float32)
            zk_sb = cpool.tile([128, 2, C], dt.float32)
            for bt in range(2):
                nc.sync.dma_start(zq_sb[:, bt, :], zq_d[bt])
                nc.sync.dma_start(zk_sb[:, bt, :], zk_d[bt])

            # ---- l_pos = sum(Z_q * Z_k, axis=-1) ----
            if lpos:
                prod = cpool.tile([128, 2, C], dt.float32)
                nc.vector.tensor_mul(prod[:], zq_sb[:], zk_sb[:])
                lpos_sb = cpool.tile([128, 2], dt.float32)
                nc.vector.tensor_reduce(
                    lpos_sb[:],
                    prod[:],
                    axis=mybir.AxisListType.X,
                    op=mybir.AluOpType.add,
                )
                for bt in range(2):
                    nc.sync.dma_start(lpos_d[bt], lpos_sb[:, bt])

            # ---- part 1: queue shard ----
            rm_buf = cpool.tile([128, NJT], dt.float32)   # per-tile row max
            s0_buf = cpool.tile([128, NJT], dt.float32)   # s1[:, batch 0]
            # all 128 rows identical (= partition sum); avoids partition-1 PSUM,
            # which fails NEFF load on this runtime
            p1s_ps = ps1s.tile([128, B], dt.float32)

            for jt in range(NJT if p1 else 0):
                h, off = divmod(jt * 128, JW)
                s1_ps = ps1.tile([128, B], dt.float32, tag="s1")
                for s in range(KSUB):
                    nc.tensor.matmul(
                        s1_ps[:],
                        qt_sb[h][:, s, off : off + 128],
                        zqT_sb[:, s, :],
                        start=(s == 0),
                        stop=(s == KSUB - 1),
                    )
                if ones:
                    # exp((s1/T) - SHIFT) -> bf16; summed over partitions by PE
                    exp_t = wpool.tile([128, B], dt.bfloat16, tag="exp")
                    nc.scalar.activation(
                        exp_t[:],
                        s1_ps[:],
                        mybir.ActivationFunctionType.Exp,
                        bias=nshift_sb[:],
                        scale=1.0 / INFO_TEMP,
                    )
                    nc.tensor.matmul(
                        p1s_ps[:],
                        ones_sb[:],
                        exp_t[:],
                        start=(jt == 0),
                        stop=(jt == NJT - 1),
                    )
                nc.vector.tensor_reduce(
                    rm_buf[:, jt : jt + 1],
                    s1_ps[:],
                    axis=mybir.AxisListType.X,
                    op=mybir.AluOpType.max,
                )
                nc.scalar.activation(
                    s0_buf[:, jt : jt + 1],
                    s1_ps[:, 0:1],
                    mybir.ActivationFunctionType.Copy,
                )

            if p1:
                ge_buf = cpool.tile([128, NJT], dt.float32)
                nc.vector.tensor_tensor(
                    ge_buf[:], s0_buf[:], rm_buf[:], mybir.AluOpType.is_ge
                )
                acc_sb = cpool.tile([128, 1], dt.float32)
                nc.vector.tensor_reduce(
                    acc_sb[:],
                    ge_buf[:],
                    axis=mybir.AxisListType.X,
                    op=mybir.AluOpType.add,
                )
                nc.sync.dma_start(p1acc_d[:], acc_sb[:, 0])

            if p1 and ones:
                p1sum_sb = cpool.tile([128, B], dt.float32)
                nc.vector.tensor_copy(p1sum_sb[:], p1s_ps[:])
                nc.sync.dma_start(p1sum_d[:], p1sum_sb[:])

            # ---- part 2: centroid shard ----
            s2_sb = cpool.tile([128, 2, CSH_PAD], dt.float32)
            top8 = cpool.tile([128, 2, 8], dt.float32)
            idx8 = cpool.tile([128, 2, 8], dt.uint32)
            p2max_sb = cpool.tile([128, 2], dt.float32)
            p2idx_sb = cpool.tile([128, 2], dt.uint32)

            chunks = []
            off = 0
            while off < CSH_PAD:
                w = min(512, CSH_PAD - off)
                chunks.append((off, w))
                off += w

            for bt in range(2 if p2 else 0):
                for off, w in chunks:
                    s2_ps = ps2.tile([128, 512], dt.float32, tag="s2")
                    for s in range(KSUB):
                        nc.tensor.matmul(
                            s2_ps[:, :w],
                            zqT_sb[:, s, bt * 128 : (bt + 1) * 128],
                            cT_sb[:, s, off : off + w],
                            start=(s == 0),
                            stop=(s == KSUB - 1),
                        )
                    nc.vector.tensor_copy(s2_sb[:, bt, off : off + w], s2_ps[:, :w])
                nc.vector.max(top8[:, bt, :], s2_sb[:, bt, :CSH])
                nc.vector.max_index(idx8[:, bt, :], top8[:, bt, :], s2_sb[:, bt, :CSH])
                nc.vector.tensor_copy(p2max_sb[:, bt : bt + 1], top8[:, bt, 0:1])
                nc.vector.tensor_copy(p2idx_sb[:, bt : bt + 1], idx8[:, bt, 0:1])
                nc.sync.dma_start(s2_d[bt], s2_sb[:, bt, :])

            for bt in range(2 if p2 else 0):
                nc.sync.dma_start(p2max_d[bt], p2max_sb[:, bt])
                nc.sync.dma_start(p2idx_d[bt], p2idx_sb[:, bt])

    nc.compile()
    return nc


def _get_nc():
    if "nc" not in _CACHE:
        _CACHE["nc"] = _build()
    return _CACHE["nc"]


def _prep_inputs(Z_q, Z_k, queue, centroids):
    """Host-side shard prep: fp16 conversion + transpose to [C, rows]."""
    zqT = np.ascontiguousarray(Z_q.astype(np.float16).T)      # [512, 256]
    zqT = zqT.reshape(KSUB, 128, B)

    qT = np.ascontiguousarray(queue.astype(np.float16).T)     # [512, 65536]
    cT = np.ascontiguousarray(centroids.astype(np.float16).T)  # [512, 50000]

    zq_h = np.ascontiguousarray(Z_q.astype(np.float32)).reshape(2, 128, C)
    zk_h = np.ascontiguousarray(Z_k.astype(np.float32)).reshape(2, 128, C)

    JW = QSH // QCHUNK
    in_maps = []
    for i in range(NCORES):
        q_sh = qT[:, i * QSH : (i + 1) * QSH]                 # [512, 8192]
        # [QCHUNK, KSUB, 128, JW] so each chunk DMA is contiguous
        q_sh = np.ascontiguousarray(
            q_sh.reshape(KSUB, 128, QCHUNK, JW).transpose(2, 0, 1, 3)
        )
        c_sh = np.zeros((C, CSH_PAD), np.float16)
        c_sh[:, :CSH] = cT[:, i * CSH : (i + 1) * CSH]
        c_sh = np.ascontiguousarray(c_sh.reshape(KSUB, 128, CSH_PAD))
        in_maps.append(
            {"zqT": zqT, "qT": q_sh, "cT": c_sh, "zq": zq_h, "zk": zk_h}
        )
    return in_maps


def kernel(Z_q, Z_k, queue, centroids, kmeans_temp, neg_raw):
    global last_exec_time_ns
    from concourse.bass_utils import run_bass_kernel_spmd

    nc = _get_nc()
    in_maps = _prep_inputs(Z_q, Z_k, queue, centroids)

    trace = bool(int(os.environ.get("MOCO_BASS_TRACE", "0")))
    out = run_bass_kernel_spmd(nc, in_maps, core_ids=list(range(NCORES)), trace=trace)
    last_exec_time_ns = out.exec_time_ns
    res = out.results

    # ---- host combine (tiny) ----
    lp = res[0]["lpos"].reshape(B).astype(np.float64)         # l_pos
    lp_t = lp / INFO_TEMP

    # part-1 loss: logsumexp over [l_pos | l_neg]/T per batch row
    S = np.zeros(B, np.float64)
    for r in res:
        S += r["p1sum"][0].astype(np.float64)
    S += np.exp(lp_t - SHIFT)
    lse1 = np.log(S) + SHIFT
    loss1 = np.mean(lse1 - lp_t)

    # accuracy: count of columns where batch row 0 attains the max
    count = sum(float(r["p1acc"].sum()) for r in res)
    count += float(lp[0] >= lp.max())
    accuracy = count / (1 + QUEUE)

    # part-2: global argmax over centroids (== argmin of ||c||^2 - 2 s)
    vals = np.stack([r["p2max"].reshape(B) for r in res])      # [8, B]
    idxs = np.stack(
        [r["p2idx"].reshape(B).astype(np.int64) + i * CSH for i, r in enumerate(res)]
    )
    best = vals.max(axis=0)
    masked = np.where(vals >= best[None, :], idxs, np.iinfo(np.int64).max)
    I = masked.min(axis=0)                                      # first-index tiebreak
    maxv = best.astype(np.float64)

    s2_full = np.empty((B, NCL), np.float32)
    for i, r in enumerate(res):
        s2_full[:, i * CSH : (i + 1) * CSH] = r["s2"].reshape(B, CSH_PAD)[:, :CSH]

    kt = kmeans_temp.astype(np.float64)
    pl_pos = maxv / kt[I]                                       # [B]
    neg_idx = neg_raw + (neg_raw >= I[:, None]).astype(neg_raw.dtype)
    pl_neg = (
        np.take_along_axis(s2_full, neg_idx, axis=1).astype(np.float64)
        / kt[neg_idx]
    )
    plogits = np.concatenate([pl_pos[:, None], pl_neg], axis=1)
    m = plogits.max(axis=1)
    plse = np.log(np.exp(plogits - m[:, None]).sum(axis=1)) + m
    ploss = np.mean(plse - pl_pos)

    loss = loss1 + PROTO_FACTOR * ploss
    return np.float32(loss), np.float32(accuracy)


# revision 17
# speedup vs baseline: 1.0727x; 1.0727x over previous
"""MoCo loss (InfoNCE over a 65536-entry queue + proto-NCE over 50000
k-means centroids) on 8 Trainium2 NeuronCores.

Strategy: the heavy work is two matmuls, Z_q @ queue.T (256x512x65536)
and Z_q @ centroids.T (256x512x50000).  We shard the tables by row
across the 8 cores and replicate Z_q.  The host pre-transposes the
tables to [C, rows] (fp16) so both matmul operands have the contraction
dim C on partitions.  Per core the device computes:

  part 2 first (centroid shard, layout [batch -> partitions, centroids -> free]):
    - s2 = Z_q @ c_shard.T                      (PE, fp16 in / fp32 acc)
    - top-1 value+index per batch row           (DVE max/max_index)
      (centroids are L2-normalized so argmin ||c||^2 - 2 s == argmax s)
    - s2 shard exported to DRAM as fp16         (gpsimd cast-DMA)
  part 1 (queue shard, layout [queue rows -> partitions, batch -> free]):
    - s1 = q_shard @ Z_q.T                      (PE)
    - row-max over batch + (s1[:,0] >= rowmax)  -> accuracy counts (DVE)
    - exp(s1/T - SHIFT)                         (ACT)
    - ones-matmul partition sum                 -> per-batch partial
                                                   sum-of-exp (PE)

The host combines the tiny per-core partials (logsumexp merge, accuracy
count, global argmax) and does the 256x512 exclusion gather + 513-wide
softmax for the proto term (0.003% of total FLOPs).
"""

import os
import numpy as np

B, C = 256, 512
QUEUE, NCL, NNEG = 65536, 50000, 512
INFO_TEMP = 0.07
PROTO_FACTOR = 0.5
NCORES = 8
QSH = QUEUE // NCORES          # 8192 queue rows per core
CSH = NCL // NCORES            # 6250 centroid rows per core
CSH_PAD = 6272                 # 14 * 448
CCH = 14                       # cT DMA/matmul chunks
CW = CSH_PAD // CCH            # 448
KSUB = C // 128                # 4 contraction subtiles
SHIFT = 14.0                   # logits <= 1/0.07 = 14.2857
NBT = 16                       # part-1 big tiles (512 queue rows each)
BTW = QSH // NBT               # 512 queue rows per big tile = 4 j-subtiles
QCHUNK = 8                     # qT DMA chunks (1 MiB each)
JW = QSH // QCHUNK             # 1024

_CACHE = {}

# exec time of the last device run (ns), populated when tracing is on
last_exec_time_ns = None


def _build():
    import concourse.bass as bass
    import concourse.tile as tile
    from concourse import bacc, mybir

    dt = mybir.dt
    nc = bacc.Bacc(
        "TRN2", target_bir_lowering=False, debug=False, num_devices=NCORES
    )

    # ---- DRAM I/O ----
    zqT_d = nc.dram_tensor("zqT", [KSUB, 128, B], dt.float16, kind="ExternalInput").ap()
    qT_d = nc.dram_tensor(
        "qT", [QCHUNK, KSUB, 128, JW], dt.float16, kind="ExternalInput"
    ).ap()
    cT_d = nc.dram_tensor(
        "cT", [CCH, KSUB, 128, CW], dt.float16, kind="ExternalInput"
    ).ap()
    zq_d = nc.dram_tensor("zq", [2, 128, C], dt.float32, kind="ExternalInput").ap()
    zk_d = nc.dram_tensor("zk", [2, 128, C], dt.float32, kind="ExternalInput").ap()

    # [128, B]: all rows identical (partition-1 DMAs fail NEFF load here)
    p1sum_d = nc.dram_tensor("p1sum", [128, B], dt.float32, kind="ExternalOutput").ap()
    p1acc_d = nc.dram_tensor("p1acc", [128], dt.float32, kind="ExternalOutput").ap()
    s2_d = nc.dram_tensor(
        "s2", [2, CCH, 128, CW], dt.float16, kind="ExternalOutput"
    ).ap()
    p2max_d = nc.dram_tensor("p2max", [2, 128], dt.float32, kind="ExternalOutput").ap()
    p2idx_d = nc.dram_tensor("p2idx", [2, 128], dt.uint32, kind="ExternalOutput").ap()
    lpos_d = nc.dram_tensor("lpos", [2, 128], dt.float32, kind="ExternalOutput").ap()

    with tile.TileContext(nc) as tc:
        with (
            tc.tile_pool(name="const", bufs=1) as cpool,
            tc.tile_pool(name="work", bufs=3) as wpool,
            tc.tile_pool(name="ps1", bufs=2, space="PSUM") as ps1,
            tc.tile_pool(name="psum1s", bufs=1, space="PSUM") as ps1s,
            tc.tile_pool(name="ps2", bufs=2, space="PSUM") as ps2,
        ):
            # ---- resident SBUF tensors (DMA issue order matters) ----
            zqT_sb = cpool.tile([128, KSUB, B], dt.float16)
            for s in range(KSUB):
                nc.sync.dma_start(zqT_sb[:, s, :], zqT_d[s])

            cT_sb = cpool.tile([128, CCH, KSUB, CW], dt.float16)
            for ch in range(4):
                nc.sync.dma_start(
                    cT_sb[:, ch], cT_d[ch].rearrange("s c j -> c s j")
                )

            qt_sb = []
            for h in range(QCHUNK):
                t = cpool.tile([128, KSUB, JW], dt.float16, tag=f"qt{h}")
                qt_sb.append(t)

            # interleave the rest of cT with the first qT chunks
            for ch in range(4, CCH):
                nc.sync.dma_start(
                    cT_sb[:, ch], cT_d[ch].rearrange("s c j -> c s j")
                )
                if ch - 4 < QCHUNK:
                    nc.sync.dma_start(
                        qt_sb[ch - 4][:], qT_d[ch - 4].rearrange("s c j -> c s j")
                    )
            for h in range(CCH - 4, QCHUNK):
                nc.sync.dma_start(qt_sb[h][:], qT_d[h].rearrange("s c j -> c s j"))

            zq_sb = cpool.tile([128, 2, C], dt.float32)
            zk_sb = cpool.tile([128, 2, C], dt.float32)
            for bt in range(2):
                nc.sync.dma_start(zq_sb[:, bt, :], zq_d[bt])
                nc.sync.dma_start(zk_sb[:, bt, :], zk_d[bt])

            ones_sb = cpool.tile([128, 128], dt.bfloat16)
            nc.vector.memset(ones_sb[:], 1.0)
            nshift_sb = cpool.tile([128, 1], dt.float32)
            nc.vector.memset(nshift_sb[:], -SHIFT)

            # ---- part 2: centroid shard ----
            s2_sb = cpool.tile([128, 2, CCH, CW], dt.float32)
            top8 = cpool.tile([128, 2, 8], dt.float32)
            idx8 = cpool.tile([128, 2, 8], dt.uint32)
            p2max_sb = cpool.tile([128, 2], dt.float32)
            p2idx_sb = cpool.tile([128, 2], dt.uint32)

            for bt in range(2):
                for ch in range(CCH):
                    s2_ps = ps2.tile([128, CW], dt.float32, tag="s2")
                    for s in range(KSUB):
                        nc.tensor.matmul(
                            s2_ps[:],
                            zqT_sb[:, s, bt * 128 : (bt + 1) * 128],
                            cT_sb[:, ch, s, :],
                            start=(s == 0),
                            stop=(s == KSUB - 1),
                        )
                    nc.scalar.activation(
                        s2_sb[:, bt, ch, :],
                        s2_ps[:],
                        mybir.ActivationFunctionType.Copy,
                    )
                    # fp16 export (cast during SWDGE DMA)
                    nc.gpsimd.dma_start(s2_d[bt, ch], s2_sb[:, bt, ch, :])
                nc.vector.max(
                    top8[:, bt, :], s2_sb[:, bt].rearrange("p c w -> p (c w)")[:, :CSH]
                )
                nc.vector.max_index(
                    idx8[:, bt, :],
                    top8[:, bt, :],
                    s2_sb[:, bt].rearrange("p c w -> p (c w)")[:, :CSH],
                )
                nc.vector.tensor_copy(p2max_sb[:, bt : bt + 1], top8[:, bt, 0:1])
                nc.vector.tensor_copy(p2idx_sb[:, bt : bt + 1], idx8[:, bt, 0:1])

            for bt in range(2):
                nc.sync.dma_start(p2max_d[bt], p2max_sb[:, bt])
                nc.sync.dma_start(p2idx_d[bt], p2idx_sb[:, bt])

            # ---- l_pos = sum(Z_q * Z_k, axis=-1) ----
            prod = cpool.tile([128, 2, C], dt.float32)
            nc.vector.tensor_mul(prod[:], zq_sb[:], zk_sb[:])
            lpos_sb = cpool.tile([128, 2], dt.float32)
            nc.vector.tensor_reduce(
                lpos_sb[:], prod[:], axis=mybir.AxisListType.X, op=mybir.AluOpType.add
            )
            for bt in range(2):
                nc.sync.dma_start(lpos_d[bt], lpos_sb[:, bt])

            # ---- part 1: queue shard, 16 big tiles of 512 rows ----
            rm_buf = cpool.tile([128, NBT, 4], dt.float32)  # per-j-subtile row max
            s0_buf = cpool.tile([128, NBT, 4], dt.float32)  # s1[:, batch 0]
            p1s_ps = ps1s.tile([128, 2, 512], dt.float32)   # sum-of-exp accumulator

            for t in range(NBT):
                s1_ps = ps1.tile([128, 4, B], dt.float32, tag="s1")
                for q in range(4):
                    jt = t * 4 + q
                    h, off = divmod(jt * 128, JW)
                    for s in range(KSUB):
                        nc.tensor.matmul(
                            s1_ps[:, q, :],
                            qt_sb[h][:, s, off : off + 128],
                            zqT_sb[:, s, :],
                            start=(s == 0),
                            stop=(s == KSUB - 1),
                        )
                exp_t = wpool.tile([128, 4, B], dt.bfloat16, tag="exp")
                nc.scalar.activation(
                    exp_t[:],
                    s1_ps[:],
                    mybir.ActivationFunctionType.Exp,
                    bias=nshift_sb[:],
                    scale=1.0 / INFO_TEMP,
                )
                for hh in range(2):
                    nc.tensor.matmul(
                        p1s_ps[:, hh, :],
                        ones_sb[:],
                        exp_t[:].rearrange("p q b -> p (q b)")[
                            :, hh * 512 : (hh + 1) * 512
                        ],
                        start=(t == 0),
                        stop=(t == NBT - 1),
                    )
                nc.vector.tensor_reduce(
                    rm_buf[:, t, :],
                    s1_ps[:],
                    axis=mybir.AxisListType.X,
                    op=mybir.AluOpType.max,
                )
                nc.scalar.activation(
                    s0_buf[:, t, :],
                    s1_ps[:, :, 0],
                    mybir.ActivationFunctionType.Copy,
                )

            ge_buf = cpool.tile([128, NBT * 4], dt.float32)
            nc.vector.tensor_tensor(
                ge_buf[:],
                s0_buf[:].rearrange("p t q -> p (t q)"),
                rm_buf[:].rearrange("p t q -> p (t q)"),
                mybir.AluOpType.is_ge,
            )
            acc_sb = cpool.tile([128, 1], dt.float32)
            nc.vector.tensor_reduce(
                acc_sb[:], ge_buf[:], axis=mybir.AxisListType.X, op=mybir.AluOpType.add
            )
            nc.sync.dma_start(p1acc_d[:], acc_sb[:, 0])

            # sum-of-exp: copy PSUM accumulator, fold the 4 quarters
            p1s_sb = cpool.tile([128, 4, B], dt.float32)
            nc.vector.tensor_copy(
                p1s_sb[:], p1s_ps[:].rearrange("p h x -> p (h x)")
            )
            p1sum_sb = cpool.tile([128, B], dt.float32)
            nc.vector.tensor_reduce(
                p1sum_sb[:],
                p1s_sb[:].rearrange("p q b -> p b q"),
                axis=mybir.AxisListType.X,
                op=mybir.AluOpType.add,
            )
            nc.sync.dma_start(p1sum_d[:], p1sum_sb[:])

    nc.compile()
    return nc


def _get_nc():
    if "nc" not in _CACHE:
        _CACHE["nc"] = _build()
    return _CACHE["nc"]


def _prep_inputs(Z_q, Z_k, queue, centroids):
    """Host-side shard prep: fp16 conversion + transpose to [C, rows]."""
    zqT = np.ascontiguousarray(Z_q.astype(np.float16).T)      # [512, 256]
    zqT = zqT.reshape(KSUB, 128, B)

    qT = np.ascontiguousarray(queue.astype(np.float16).T)     # [512, 65536]
    cT = np.ascontiguousarray(centroids.astype(np.float16).T)  # [512, 50000]

    zq_h = np.ascontiguousarray(Z_q.astype(np.float32)).reshape(2, 128, C)
    zk_h = np.ascontiguousarray(Z_k.astype(np.float32)).reshape(2, 128, C)

    in_maps = []
    for i in range(NCORES):
        q_sh = qT[:, i * QSH : (i + 1) * QSH]                 # [512, 8192]
        # [QCHUNK, KSUB, 128, JW] so each chunk DMA is contiguous
        q_sh = np.ascontiguousarray(
            q_sh.reshape(KSUB, 128, QCHUNK, JW).transpose(2, 0, 1, 3)
        )
        c_sh = np.zeros((C, CSH_PAD), np.float16)
        c_sh[:, :CSH] = cT[:, i * CSH : (i + 1) * CSH]
        # [CCH, KSUB, 128, CW] chunk-major
        c_sh = np.ascontiguousarray(
            c_sh.reshape(KSUB, 128, CCH, CW).transpose(2, 0, 1, 3)
        )
        in_maps.append(
            {"zqT": zqT, "qT": q_sh, "cT": c_sh, "zq": zq_h, "zk": zk_h}
        )
    return in_maps


def kernel(Z_q, Z_k, queue, centroids, kmeans_temp, neg_raw):
    global last_exec_time_ns
    from concourse.bass_utils import run_bass_kernel_spmd

    nc = _get_nc()
    in_maps = _prep_inputs(Z_q, Z_k, queue, centroids)

    trace = bool(int(os.environ.get("MOCO_BASS_TRACE", "0")))
    out = run_bass_kernel_spmd(nc, in_maps, core_ids=list(range(NCORES)), trace=trace)
    last_exec_time_ns = out.exec_time_ns
    res = out.results

    # ---- host combine (tiny) ----
    lp = res[0]["lpos"].reshape(B).astype(np.float64)         # l_pos
    lp_t = lp / INFO_TEMP

    # part-1 loss: logsumexp over [l_pos | l_neg]/T per batch row
    S = np.zeros(B, np.float64)
    for r in res:
        S += r["p1sum"][0].astype(np.float64)
    S += np.exp(lp_t - SHIFT)
    lse1 = np.log(S) + SHIFT
    loss1 = np.mean(lse1 - lp_t)

    # accuracy: count of columns where batch row 0 attains the max
    count = sum(float(r["p1acc"].sum()) for r in res)
    count += float(lp[0] >= lp.max())
    accuracy = count / (1 + QUEUE)

    # part-2: global argmax over centroids (== argmin of ||c||^2 - 2 s)
    vals = np.stack([r["p2max"].reshape(B) for r in res])      # [8, B]
    idxs = np.stack(
        [r["p2idx"].reshape(B).astype(np.int64) + i * CSH for i, r in enumerate(res)]
    )
    best = vals.max(axis=0)
    masked = np.where(vals >= best[None, :], idxs, np.iinfo(np.int64).max)
    I = masked.min(axis=0)                                      # first-index tiebreak
    maxv = best.astype(np.float64)

    s2_full = np.empty((B, NCL), np.float16)
    for i, r in enumerate(res):
        sh = r["s2"].transpose(0, 2, 1, 3).reshape(B, CSH_PAD)  # [2,CCH,128,CW]
        s2_full[:, i * CSH : (i + 1) * CSH] = sh[:, :CSH]

    kt = kmeans_temp.astype(np.float64)
    pl_pos = maxv / kt[I]                                       # [B]
    neg_idx = neg_raw + (neg_raw >= I[:, None]).astype(neg_raw.dtype)
    pl_neg = (
        np.take_along_axis(s2_full, neg_idx, axis=1).astype(np.float64)
        / kt[neg_idx]
    )
    plogits = np.concatenate([pl_pos[:, None], pl_neg], axis=1)
    m = plogits.max(axis=1)
    plse = np.log(np.exp(plogits - m[:, None]).sum(axis=1)) + m
    ploss = np.mean(plse - pl_pos)

    loss = loss1 + PROTO_FACTOR * ploss
    return np.float32(loss), np.float32(accuracy)


# revision 24
# speedup vs baseline: 1.3322x; 1.2419x over previous
"""MoCo loss (InfoNCE over a 65536-entry queue + proto-NCE over 50000
k-means centroids) on 8 Trainium2 NeuronCores.

Strategy: the heavy work is two matmuls, Z_q @ queue.T (256x512x65536)
and Z_q @ centroids.T (256x512x50000).  We shard the tables by row
across the 8 cores and replicate Z_q.  The host pre-transposes the
tables to [C, rows] (fp16) so both matmul operands have the contraction
dim C on partitions.  Per core the device computes:

  part 2 first (centroid shard, layout [batch -> partitions, centroids -> free]):
    - s2 = Z_q @ c_shard.T                      (PE, fp16 in / fp32 acc)
    - top-1 value+index per batch row           (DVE max/max_index)
      (centroids are L2-normalized so argmin ||c||^2 - 2 s == argmax s)
    - s2 shard exported to DRAM as fp16         (gpsimd cast-DMA)
  part 1 (queue shard, layout [queue rows -> partitions, batch -> free]):
    - s1 = q_shard @ Z_q.T                      (PE)
    - row-max over batch + (s1[:,0] >= rowmax)  -> accuracy counts (DVE)
    - exp(s1/T - SHIFT)                         (ACT)
    - ones-matmul partition sum                 -> per-batch partial
                                                   sum-of-exp (PE)

The host combines the tiny per-core partials (logsumexp merge, accuracy
count, global argmax) and does the 256x512 exclusion gather + 513-wide
softmax for the proto term (0.003% of total FLOPs).
"""

import os
import numpy as np

B, C = 256, 512
QUEUE, NCL, NNEG = 65536, 50000, 512
INFO_TEMP = 0.07
PROTO_FACTOR = 0.5
NCORES = 8
QSH = QUEUE // NCORES          # 8192 queue rows per core
CSH = NCL // NCORES            # 6250 centroid rows per core
CSH_PAD = 6272                 # 14 * 448
CCH = 14                       # cT DMA/matmul chunks
CW = CSH_PAD // CCH            # 448
KSUB = C // 128                # 4 contraction subtiles
SHIFT = 14.0                   # logits <= 1/0.07 = 14.2857
NBT = 16                       # part-1 big tiles (512 queue rows each)
BTW = QSH // NBT               # 512 queue rows per big tile = 4 j-subtiles
QCHUNK = 8                     # qT DMA chunks (1 MiB each)
JW = QSH // QCHUNK             # 1024

_CACHE = {}

# exec time of the last device run (ns), populated when tracing is on
last_exec_time_ns = None


def _build():
    import concourse.bass as bass
    import concourse.tile as tile
    from concourse import bacc, mybir

    dt = mybir.dt
    nc = bacc.Bacc(
        "TRN2", target_bir_lowering=False, debug=False, num_devices=NCORES
    )

    # ---- DRAM I/O ----
    zqT_d = nc.dram_tensor("zqT", [KSUB, 128, B], dt.float16, kind="ExternalInput").ap()
    qT_d = nc.dram_tensor(
        "qT", [QCHUNK, KSUB, 128, JW], dt.float16, kind="ExternalInput"
    ).ap()
    cT_d = nc.dram_tensor(
        "cT", [CCH, KSUB, 128, CW], dt.float16, kind="ExternalInput"
    ).ap()
    zq_d = nc.dram_tensor("zq", [2, 128, C], dt.float32, kind="ExternalInput").ap()
    zk_d = nc.dram_tensor("zk", [2, 128, C], dt.float32, kind="ExternalInput").ap()

    # [128, B]: all rows identical (partition-1 DMAs fail NEFF load here)
    p1sum_d = nc.dram_tensor("p1sum", [128, B], dt.float32, kind="ExternalOutput").ap()
    p1acc_d = nc.dram_tensor("p1acc", [128], dt.float32, kind="ExternalOutput").ap()
    s2_d = nc.dram_tensor(
        "s2", [2, CCH, 128, CW], dt.float16, kind="ExternalOutput"
    ).ap()
    lpos_d = nc.dram_tensor("lpos", [2, 128], dt.float32, kind="ExternalOutput").ap()

    with tile.TileContext(nc) as tc:
        with (
            tc.tile_pool(name="const", bufs=1) as cpool,
            tc.tile_pool(name="work", bufs=3) as wpool,
            tc.tile_pool(name="ps1", bufs=2, space="PSUM") as ps1,
            tc.tile_pool(name="psum1s", bufs=1, space="PSUM") as ps1s,
            tc.tile_pool(name="ps2", bufs=3, space="PSUM") as ps2,
        ):
            # ---- resident SBUF tensors (DMA issue order matters) ----
            zqT_sb = cpool.tile([128, KSUB, B], dt.float16)
            for s in range(KSUB):
                nc.sync.dma_start(zqT_sb[:, s, :], zqT_d[s])

            cT_sb = cpool.tile([128, CCH, KSUB, CW], dt.float16)
            for ch in range(4):
                nc.sync.dma_start(
                    cT_sb[:, ch], cT_d[ch].rearrange("s c j -> c s j")
                )

            qt_sb = []
            for h in range(QCHUNK):
                t = cpool.tile([128, KSUB, JW], dt.float16, tag=f"qt{h}")
                qt_sb.append(t)

            # interleave the rest of cT with the first qT chunks
            for ch in range(4, CCH):
                nc.sync.dma_start(
                    cT_sb[:, ch], cT_d[ch].rearrange("s c j -> c s j")
                )
                if ch - 4 < QCHUNK:
                    nc.sync.dma_start(
                        qt_sb[ch - 4][:], qT_d[ch - 4].rearrange("s c j -> c s j")
                    )
            for h in range(CCH - 4, QCHUNK):
                nc.sync.dma_start(qt_sb[h][:], qT_d[h].rearrange("s c j -> c s j"))

            zq_sb = cpool.tile([128, 2, C], dt.float32)
            zk_sb = cpool.tile([128, 2, C], dt.float32)
            for bt in range(2):
                nc.sync.dma_start(zq_sb[:, bt, :], zq_d[bt])
                nc.sync.dma_start(zk_sb[:, bt, :], zk_d[bt])

            ones_sb = cpool.tile([128, 128], dt.bfloat16)
            nc.vector.memset(ones_sb[:], 1.0)
            nshift_sb = cpool.tile([128, 1], dt.float32)
            nc.vector.memset(nshift_sb[:], -SHIFT)

            # ---- part 2: centroid shard (argmax happens on the host) ----
            s2_sb = cpool.tile([128, 2, CCH, CW], dt.float16)

            for bt in range(2):
                for ch in range(CCH):
                    s2_ps = ps2.tile([128, CW], dt.float32, tag="s2")
                    for s in range(KSUB):
                        nc.tensor.matmul(
                            s2_ps[:],
                            zqT_sb[:, s, bt * 128 : (bt + 1) * 128],
                            cT_sb[:, ch, s, :],
                            start=(s == 0),
                            stop=(s == KSUB - 1),
                        )
                    # cast to fp16; alternate ACT/DVE to balance engines
                    if ch % 2 == 0:
                        nc.scalar.activation(
                            s2_sb[:, bt, ch, :],
                            s2_ps[:],
                            mybir.ActivationFunctionType.Copy,
                        )
                    else:
                        nc.vector.tensor_copy(s2_sb[:, bt, ch, :], s2_ps[:])
                    nc.sync.dma_start(s2_d[bt, ch], s2_sb[:, bt, ch, :])

            # ---- l_pos = sum(Z_q * Z_k, axis=-1) ----
            prod = cpool.tile([128, 2, C], dt.float32)
            nc.vector.tensor_mul(prod[:], zq_sb[:], zk_sb[:])
            lpos_sb = cpool.tile([128, 2], dt.float32)
            nc.vector.tensor_reduce(
                lpos_sb[:], prod[:], axis=mybir.AxisListType.X, op=mybir.AluOpType.add
            )
            for bt in range(2):
                nc.sync.dma_start(lpos_d[bt], lpos_sb[:, bt])

            # ---- part 1: queue shard, 16 big tiles of 512 rows ----
            rm_buf = cpool.tile([128, NBT, 4], dt.float32)  # per-j-subtile row max
            s0_buf = cpool.tile([128, NBT, 4], dt.float32)  # s1[:, batch 0]
            p1s_ps = ps1s.tile([128, B], dt.float32)        # sum-of-exp accumulator

            for t in range(NBT):
                s1_ps = ps1.tile([128, 4, B], dt.float32, tag="s1")
                for q in range(4):
                    jt = t * 4 + q
                    h, off = divmod(jt * 128, JW)
                    for s in range(KSUB):
                        nc.tensor.matmul(
                            s1_ps[:, q, :],
                            qt_sb[h][:, s, off : off + 128],
                            zqT_sb[:, s, :],
                            start=(s == 0),
                            stop=(s == KSUB - 1),
                        )
                exp_t = wpool.tile([128, 4, B], dt.bfloat16, tag="exp")
                nc.scalar.activation(
                    exp_t[:],
                    s1_ps[:],
                    mybir.ActivationFunctionType.Exp,
                    bias=nshift_sb[:],
                    scale=1.0 / INFO_TEMP,
                )
                for q in range(4):
                    nc.tensor.matmul(
                        p1s_ps[:],
                        ones_sb[:],
                        exp_t[:, q, :],
                        start=(t == 0 and q == 0),
                        stop=(t == NBT - 1 and q == 3),
                    )
                nc.vector.tensor_reduce(
                    rm_buf[:, t, :],
                    s1_ps[:],
                    axis=mybir.AxisListType.X,
                    op=mybir.AluOpType.max,
                )
                nc.scalar.activation(
                    s0_buf[:, t, :],
                    s1_ps[:, :, 0],
                    mybir.ActivationFunctionType.Copy,
                )

            ge_buf = cpool.tile([128, NBT * 4], dt.float32)
            nc.vector.tensor_tensor(
                ge_buf[:],
                s0_buf[:].rearrange("p t q -> p (t q)"),
                rm_buf[:].rearrange("p t q -> p (t q)"),
                mybir.AluOpType.is_ge,
            )
            acc_sb = cpool.tile([128, 1], dt.float32)
            nc.vector.tensor_reduce(
                acc_sb[:], ge_buf[:], axis=mybir.AxisListType.X, op=mybir.AluOpType.add
            )
            nc.sync.dma_start(p1acc_d[:], acc_sb[:, 0])

            p1sum_sb = cpool.tile([128, B], dt.float32)
            nc.vector.tensor_copy(p1sum_sb[:], p1s_ps[:])
            nc.sync.dma_start(p1sum_d[:], p1sum_sb[:])

    nc.compile()
    return nc


def _get_nc():
    if "nc" not in _CACHE:
        _CACHE["nc"] = _build()
    return _CACHE["nc"]


def _prep_inputs(Z_q, Z_k, queue, centroids):
    """Host-side shard prep: fp16 conversion + transpose to [C, rows]."""
    zqT = np.ascontiguousarray(Z_q.astype(np.float16).T)      # [512, 256]
    zqT = zqT.reshape(KSUB, 128, B)

    qT = np.ascontiguousarray(queue.astype(np.float16).T)     # [512, 65536]
    cT = np.ascontiguousarray(centroids.astype(np.float16).T)  # [512, 50000]

    zq_h = np.ascontiguousarray(Z_q.astype(np.float32)).reshape(2, 128, C)
    zk_h = np.ascontiguousarray(Z_k.astype(np.float32)).reshape(2, 128, C)

    in_maps = []
    for i in range(NCORES):
        q_sh = qT[:, i * QSH : (i + 1) * QSH]                 # [512, 8192]
        # [QCHUNK, KSUB, 128, JW] so each chunk DMA is contiguous
        q_sh = np.ascontiguousarray(
            q_sh.reshape(KSUB, 128, QCHUNK, JW).transpose(2, 0, 1, 3)
        )
        c_sh = np.zeros((C, CSH_PAD), np.float16)
        c_sh[:, :CSH] = cT[:, i * CSH : (i + 1) * CSH]
        # [CCH, KSUB, 128, CW] chunk-major
        c_sh = np.ascontiguousarray(
            c_sh.reshape(KSUB, 128, CCH, CW).transpose(2, 0, 1, 3)
        )
        in_maps.append(
            {"zqT": zqT, "qT": q_sh, "cT": c_sh, "zq": zq_h, "zk": zk_h}
        )
    return in_maps


def kernel(Z_q, Z_k, queue, centroids, kmeans_temp, neg_raw):
    global last_exec_time_ns
    from concourse.bass_utils import run_bass_kernel_spmd

    nc = _get_nc()
    in_maps = _prep_inputs(Z_q, Z_k, queue, centroids)

    trace = bool(int(os.environ.get("MOCO_BASS_TRACE", "0")))
    out = run_bass_kernel_spmd(nc, in_maps, core_ids=list(range(NCORES)), trace=trace)
    last_exec_time_ns = out.exec_time_ns
    res = out.results

    # ---- host combine (tiny) ----
    lp = res[0]["lpos"].reshape(B).astype(np.float64)         # l_pos
    lp_t = lp / INFO_TEMP

    # part-1 loss: logsumexp over [l_pos | l_neg]/T per batch row
    S = np.zeros(B, np.float64)
    for r in res:
        S += r["p1sum"][0].astype(np.float64)
    S += np.exp(lp_t - SHIFT)
    lse1 = np.log(S) + SHIFT
    loss1 = np.mean(lse1 - lp_t)

    # accuracy: count of columns where batch row 0 attains the max
    count = sum(float(r["p1acc"].sum()) for r in res)
    count += float(lp[0] >= lp.max())
    accuracy = count / (1 + QUEUE)

    # part-2: global argmax over centroids (== argmin of ||c||^2 - 2 s)
    s2_full = np.empty((B, NCL), np.float16)
    for i, r in enumerate(res):
        sh = r["s2"].transpose(0, 2, 1, 3).reshape(B, CSH_PAD)  # [2,CCH,128,CW]
        s2_full[:, i * CSH : (i + 1) * CSH] = sh[:, :CSH]

    I = np.argmax(s2_full, axis=1)                              # first-index ties
    maxv = s2_full[np.arange(B), I].astype(np.float64)

    kt = kmeans_temp.astype(np.float64)
    pl_pos = maxv / kt[I]                                       # [B]
    neg_idx = neg_raw + (neg_raw >= I[:, None]).astype(neg_raw.dtype)
    pl_neg = (
        np.take_along_axis(s2_full, neg_idx, axis=1).astype(np.float64)
        / kt[neg_idx]
    )
    plogits = np.concatenate([pl_pos[:, None], pl_neg], axis=1)
    m = plogits.max(axis=1)
    plse = np.log(np.exp(plogits - m[:, None]).sum(axis=1)) + m
    ploss = np.mean(plse - pl_pos)

    loss = loss1 + PROTO_FACTOR * ploss
    return np.float32(loss), np.float32(accuracy)
